# revision 12
# baseline (speedup 1.0000x reference)
"""Additive attention kernel for Trainium2, 8 NeuronCores, data-parallel.

Problem (hardcoded shapes):
    query (4, 512, 256), key (4, 512, 256), value (4, 512, 256)
    W_q (256, 128), W_k (256, 128), W_v (128,)
    out = softmax_j( sum_h W_v[h] * tanh(q[b,i,h] + k[b,j,h]) ) @ value

Sharding: 8 cores = 4 batches x 2 query-halves. Each core computes 256
queries x 512 keys fully locally (no collectives).

Per-core dataflow:
    setup:  PE-transpose query/key chunks, project with W_q/W_k to get
            qT [h=128, i=256] and kT [h=128, j=512] in SBUF (h on partitions).
    main:   per 16-query block: DVE tensor_scalar_add builds
            s[h, ii*512+j] = kT[h,j] + qT[h,i] (fp32, 2x mode), one ACT tanh
            [128, 8192] -> fp16 feats f, then per (query, j-chunk) a PE
            matmul with the f slice [128,128] stationary and W_v moving
            writes a [128,1] column into scoresT[j%128, i] PSUM tiles.
    tail:   ACT exp on the 4 scoresT tiles -> e_T [j, i] fp16 in SBUF
            (|scores| <= sum|W_v| ~ 9, so no max subtraction needed),
            attn@V matmuls with lhsT=e_T slices and rhs=value (+ ones
            column for the softmax denominators), DVE reciprocal + per-row
            scale, DMA out.
"""

import os
from contextlib import ExitStack

import numpy as np

import concourse.bacc as bacc
import concourse.bass as bass
import concourse.tile as tile
from concourse import mybir
from concourse.bass import ts
from concourse.bass_utils import run_bass_kernel_spmd
from concourse.masks import make_identity

P = 128          # partitions
N_LOC = 256      # queries per core
M = 512          # keys per core
H = 128          # hidden
QK = 256         # Q_SIZE == K_SIZE
DV = 256         # value dim
IB = int(os.environ.get("KB_IB", "32"))   # max queries per tanh block
# Small blocks at the ends: the first tanh starts sooner after setup, and
# the last block's PE score-matmuls drain sooner before the tail.
BLOCKS = [4, 6, 10, 16, 26, 34, 32] + [32, 32, 32, 12, 8, 8, 4]
HALF_BLK = 6   # blocks 0..6 cover queries 0..127
assert sum(BLOCKS) == N_LOC and sum(BLOCKS[:HALF_BLK + 1]) == P
S_FP16 = os.environ.get("KB_S_FP16", "1") == "1"  # fp16 adds + kT/s

FP32 = mybir.dt.float32
FP16 = mybir.dt.float16
Tanh = mybir.ActivationFunctionType.Tanh
Exp = mybir.ActivationFunctionType.Exp

_NC = None
LAST_RESULT = None  # BassKernelResults of the most recent run (for test.py)


def _body(tc, q_d, k_d, v_d, wq_d, wk_d, wv_d, out_d, ctx):
    nc = tc.nc

    consts = ctx.enter_context(tc.tile_pool(name="consts", bufs=1))
    setup = ctx.enter_context(tc.tile_pool(name="setup", bufs=1))
    persist = ctx.enter_context(tc.tile_pool(name="persist", bufs=1))
    s_pool = ctx.enter_context(tc.tile_pool(name="s_pool", bufs=2))
    f_pool = ctx.enter_context(tc.tile_pool(name="f_pool", bufs=2))
    outp = ctx.enter_context(tc.tile_pool(name="outp", bufs=2))
    ps_tp = ctx.enter_context(tc.tile_pool(name="ps_tp", bufs=2, space="PSUM"))
    ps_one = ctx.enter_context(tc.tile_pool(name="ps_one", bufs=1, space="PSUM"))
    ps_sc = ctx.enter_context(tc.tile_pool(name="ps_sc", bufs=1, space="PSUM"))

    # --- constants ---
    ident = consts.tile([P, P], FP32, name="ident")
    make_identity(nc, ident)

    # Warm the ACT table set (exp_and_others contains tanh+exp) early so the
    # ~2.7us table load overlaps setup DMA instead of stalling block 0.
    warm = consts.tile([P, 2], FP32, name="warm")
    nc.vector.memset(warm, 0.0)
    nc.scalar.activation(out=warm, in_=warm, func=Tanh)

    # --- stage inputs; kn/qn gate the setup. Chunked DMAs as separate
    # tiles across both HWDGE rings so each transpose starts as soon as its
    # own chunk lands ---
    kr = k_d.rearrange("(c j) k -> j c k", c=4)
    kn = []
    for cj in range(4):
        t = setup.tile([P, QK], FP32, name=f"kn{cj}", tag=f"kn{cj}")
        (nc.sync if cj % 2 == 0 else nc.scalar).dma_start(out=t, in_=kr[:, cj])
        kn.append(t)
    qr = q_d.rearrange("(c i) k -> i c k", c=2)
    qn = []
    for ci in range(2):
        t = setup.tile([P, QK], FP32, name=f"qn{ci}", tag=f"qn{ci}")
        (nc.sync if ci % 2 == 0 else nc.scalar).dma_start(out=t, in_=qr[:, ci])
        qn.append(t)

    # --- transpose key to keyT [k, cj*128+j]; copies alternate DVE/ACT ---
    keyT = setup.tile([P, 2, M], FP16, name="keyT")  # [k, ck, j]
    for n, (cj, kc) in enumerate([(c, k) for c in range(4) for k in range(2)]):
        tp = ps_tp.tile([P, P], FP32, name="tp", tag="tp")
        nc.tensor.transpose(tp, kn[cj][:, ts(kc, P)], ident)
        if n % 2 == 0:
            nc.vector.tensor_copy(out=keyT[:, kc, ts(cj, P)], in_=tp)
        else:
            nc.scalar.copy(out=keyT[:, kc, ts(cj, P)], in_=tp)

    # --- weights (gpsimd SWDGE cast-DMAs; emitted late so gpsimd builds
    # ident first) ---
    wk_sb = persist.tile([P, 2, H], FP16, name="wk_sb")
    nc.gpsimd.dma_start(out=wk_sb, in_=wk_d.rearrange("(c k) h -> k c h", c=2))
    wq_sb = persist.tile([P, 2, H], FP16, name="wq_sb")
    nc.gpsimd.dma_start(out=wq_sb, in_=wq_d.rearrange("(c k) h -> k c h", c=2))
    wv_sb = persist.tile([P, 1], FP16, name="wv_sb")
    nc.gpsimd.dma_start(out=wv_sb, in_=wv_d)  # casts fp32 -> fp16

    # --- project: kT = W_k^T @ key^T  [h, j] ---
    kt_ps = ps_one.tile([P, M], FP32, name="kt_ps", tag="proj")
    for kc in range(2):
        nc.tensor.matmul(kt_ps, lhsT=wk_sb[:, kc, :], rhs=keyT[:, kc, :],
                         start=(kc == 0), stop=(kc == 1))
    sdt = FP16 if S_FP16 else FP32
    kT_sb = persist.tile([P, M], sdt, name="kT_sb")
    nc.vector.tensor_copy(out=kT_sb, in_=kt_ps)

    # --- transpose query to queryT [k, ci*128+i] ---
    queryT = setup.tile([P, 2, N_LOC], FP16, name="queryT")  # [k, ck, i]
    for n, (ci, kc) in enumerate([(c, k) for c in range(2) for k in range(2)]):
        tp = ps_tp.tile([P, P], FP32, name="tp", tag="tp")
        nc.tensor.transpose(tp, qn[ci][:, ts(kc, P)], ident)
        if n % 2 == 0:
            nc.vector.tensor_copy(out=queryT[:, kc, ts(ci, P)], in_=tp)
        else:
            nc.scalar.copy(out=queryT[:, kc, ts(ci, P)], in_=tp)

    # --- project: qT = W_q^T @ query^T  [h, i] ---
    qt_ps = ps_one.tile([P, N_LOC], FP32, name="qt_ps", tag="proj")
    for kc in range(2):
        nc.tensor.matmul(qt_ps, lhsT=wq_sb[:, kc, :], rhs=queryT[:, kc, :],
                         start=(kc == 0), stop=(kc == 1))
    # qT feeds tensor_scalar's scalar1 operand, which must be fp32
    qT_sb = persist.tile([P, N_LOC], FP32, name="qT_sb")
    nc.scalar.copy(out=qT_sb, in_=qt_ps)

    # value: cast-DMA (SWDGE) straight into fp16; ones column = softmax denom
    v_hf = persist.tile([P, 4, DV + 1], FP16, name="v_hf")
    nc.gpsimd.dma_start(out=v_hf[:, :, 0:DV],
                        in_=v_d.rearrange("(c j) d -> j c d", c=4))
    nc.vector.memset(v_hf[:, :, DV:DV + 1], 1.0)

    # --- scoresT PSUM tile: [j % 128, cj, i] (4KB/partition = 2 banks) ---
    scT = ps_sc.tile([P, 4, N_LOC], FP32, name="scT", tag="scT")

    e_T = persist.tile([P, 4, N_LOC], FP16, name="e_T")

    def finish_half(ic):
        # exp of scoresT columns [ic*128, (ic+1)*128) (no max subtraction:
        # |scores| <= sum|W_v| < 10), then attn @ value (+ones col =
        # denominators), normalize, store. For ic=0 this is emitted
        # mid-loop so it hides under the remaining tanh blocks.
        nc.scalar.activation(out=e_T[:, :, ts(ic, P)], in_=scT[:, :, ts(ic, P)],
                             func=Exp)
        o_ps = ps_one.tile([P, DV + 1], FP32, name="o_ps", tag="o_ps")
        for cj in range(4):
            nc.tensor.matmul(o_ps, lhsT=e_T[:, cj, ts(ic, P)],
                             rhs=v_hf[:, cj, :],
                             start=(cj == 0), stop=(cj == 3))
        rec = outp.tile([P, 1], FP32, name="rec", tag="rec")
        nc.vector.reciprocal(rec, o_ps[:, DV:DV + 1])
        o_sb = outp.tile([P, DV], FP32, name="o_sb", tag="o_sb")
        nc.vector.tensor_scalar_mul(out=o_sb, in0=o_ps[:, 0:DV], scalar1=rec)
        nc.sync.dma_start(out=out_d[ts(ic, P), :], in_=o_sb)

    # --- main loop: tanh features + W_v reduction ---
    i0 = 0
    for blk, nb in enumerate(BLOCKS):
        s = s_pool.tile([P, nb * M], sdt, name="s", tag="s")
        for ii in range(nb):
            i = i0 + ii
            nc.vector.tensor_scalar_add(
                out=s[:, ts(ii, M)], in0=kT_sb, scalar1=qT_sb[:, i:i + 1])
        f = f_pool.tile([P, nb * M], FP16, name="f", tag="f")
        nc.scalar.activation(out=f, in_=s, func=Tanh)
        for ii in range(nb):
            i = i0 + ii
            for cj in range(4):
                nc.tensor.matmul(
                    scT[:, cj, i:i + 1],
                    lhsT=f[:, ii * M + cj * P: ii * M + (cj + 1) * P],
                    rhs=wv_sb, start=True, stop=True)
        i0 += nb
        if blk == HALF_BLK + 1:
            finish_half(0)

    finish_half(1)


def _build_nc():
    nc = bacc.Bacc("TRN2", target_bir_lowering=False, debug=False, num_devices=8)
    q_d = nc.dram_tensor("query", [N_LOC, QK], FP32, kind="ExternalInput").ap()
    k_d = nc.dram_tensor("key", [M, QK], FP32, kind="ExternalInput").ap()
    v_d = nc.dram_tensor("value", [M, DV], FP32, kind="ExternalInput").ap()
    wq_d = nc.dram_tensor("W_q", [QK, H], FP32, kind="ExternalInput").ap()
    wk_d = nc.dram_tensor("W_k", [QK, H], FP32, kind="ExternalInput").ap()
    wv_d = nc.dram_tensor("W_v", [H, 1], FP32, kind="ExternalInput").ap()
    out_d = nc.dram_tensor("out", [N_LOC, DV], FP32, kind="ExternalOutput").ap()
    with tile.TileContext(nc) as tc:
        with ExitStack() as ctx:
            _body(tc, q_d, k_d, v_d, wq_d, wk_d, wv_d, out_d, ctx)
    nc.compile()
    return nc


def get_nc():
    global _NC
    if _NC is None:
        _NC = _build_nc()
    return _NC


def make_in_maps(query, key, value, W_q, W_k, W_v):
    query = np.ascontiguousarray(query, dtype=np.float32)
    key = np.ascontiguousarray(key, dtype=np.float32)
    value = np.ascontiguousarray(value, dtype=np.float32)
    W_q = np.ascontiguousarray(W_q, dtype=np.float32)
    W_k = np.ascontiguousarray(W_k, dtype=np.float32)
    W_v = np.ascontiguousarray(W_v, dtype=np.float32).reshape(H, 1)
    in_maps = []
    for core in range(8):
        b, half = divmod(core, 2)
        in_maps.append({
            "query": query[b, half * N_LOC:(half + 1) * N_LOC, :],
            "key": key[b],
            "value": value[b],
            "W_q": W_q,
            "W_k": W_k,
            "W_v": W_v,
        })
    return in_maps


def kernel(query, key, value, W_q, W_k, W_v):
    global LAST_RESULT
    nc = get_nc()
    in_maps = make_in_maps(query, key, value, W_q, W_k, W_v)
    trace = os.environ.get("BASS_TRACE", "") == "1"
    res = run_bass_kernel_spmd(nc, in_maps, core_ids=list(range(8)), trace=trace)
    LAST_RESULT = res
    out = np.empty((4, 512, DV), dtype=np.float32)
    for core in range(8):
        b, half = divmod(core, 2)
        out[b, half * N_LOC:(half + 1) * N_LOC, :] = res.results[core]["out"]
    return out


# revision 13
# speedup vs baseline: 1.1993x; 1.1993x over previous
"""Additive attention kernel for Trainium2, 8 NeuronCores, data-parallel.

Problem (hardcoded shapes):
    query (4, 512, 256), key (4, 512, 256), value (4, 512, 256)
    W_q (256, 128), W_k (256, 128), W_v (128,)
    out = softmax_j( sum_h W_v[h] * tanh(q[b,i,h] + k[b,j,h]) ) @ value

Sharding: 8 cores = 4 batches x 2 query-halves. Each core computes 256
queries x 512 keys fully locally (no collectives).

Per-core dataflow:
    setup:  PE-transpose query/key chunks, project with W_q/W_k to get
            qT [h=128, i=256] and kT [h=128, j=512] in SBUF (h on partitions).
    main:   per 16-query block: DVE tensor_scalar_add builds
            s[h, ii*512+j] = kT[h,j] + qT[h,i] (fp32, 2x mode), one ACT tanh
            [128, 8192] -> fp16 feats f, then per (query, j-chunk) a PE
            matmul with the f slice [128,128] stationary and W_v moving
            writes a [128,1] column into scoresT[j%128, i] PSUM tiles.
    tail:   ACT exp on the 4 scoresT tiles -> e_T [j, i] fp16 in SBUF
            (|scores| <= sum|W_v| ~ 9, so no max subtraction needed),
            attn@V matmuls with lhsT=e_T slices and rhs=value (+ ones
            column for the softmax denominators), DVE reciprocal + per-row
            scale, DMA out.
"""

import os
from contextlib import ExitStack

import numpy as np

import concourse.bacc as bacc
import concourse.bass as bass
import concourse.tile as tile
from concourse import mybir
from concourse.bass import ts
from concourse.bass_utils import run_bass_kernel_spmd
from concourse.masks import make_identity

P = 128          # partitions
N_LOC = 256      # queries per core
M = 512          # keys per core
H = 128          # hidden
QK = 256         # Q_SIZE == K_SIZE
DV = 256         # value dim
IB = int(os.environ.get("KB_IB", "32"))   # max queries per tanh block
# Small blocks at the ends: the first tanh starts sooner after setup, and
# the last block's PE score-matmuls drain sooner before the tail.
BLOCKS = [4, 6, 10, 16, 26, 34, 32] + [40, 40, 24, 12, 8, 4]
HALF_BLK = 6   # blocks 0..6 cover queries 0..127
assert sum(BLOCKS) == N_LOC and sum(BLOCKS[:HALF_BLK + 1]) == P
S_FP16 = os.environ.get("KB_S_FP16", "1") == "1"  # fp16 adds + kT/s

FP32 = mybir.dt.float32
FP16 = mybir.dt.float16
Tanh = mybir.ActivationFunctionType.Tanh
Exp = mybir.ActivationFunctionType.Exp

_NC = None
LAST_RESULT = None  # BassKernelResults of the most recent run (for test.py)


def _body(tc, q_d, k_d, v_d, wq_d, wk_d, wv_d, out_d, ctx):
    nc = tc.nc

    consts = ctx.enter_context(tc.tile_pool(name="consts", bufs=1))
    setup = ctx.enter_context(tc.tile_pool(name="setup", bufs=1))
    persist = ctx.enter_context(tc.tile_pool(name="persist", bufs=1))
    s_pool = ctx.enter_context(tc.tile_pool(name="s_pool", bufs=2))
    f_pool = ctx.enter_context(tc.tile_pool(name="f_pool", bufs=2))
    outp = ctx.enter_context(tc.tile_pool(name="outp", bufs=2))
    ps_tp = ctx.enter_context(tc.tile_pool(name="ps_tp", bufs=3, space="PSUM"))
    ps_one = ctx.enter_context(tc.tile_pool(name="ps_one", bufs=1, space="PSUM"))
    ps_sc = ctx.enter_context(tc.tile_pool(name="ps_sc", bufs=1, space="PSUM"))

    # --- constants ---
    ident = consts.tile([P, P], FP32, name="ident")
    make_identity(nc, ident)

    # Warm the ACT table set (exp_and_others contains tanh+exp) early so the
    # ~2.7us table load overlaps setup DMA instead of stalling block 0.
    warm = consts.tile([P, 2], FP32, name="warm")
    nc.vector.memset(warm, 0.0)
    nc.scalar.activation(out=warm, in_=warm, func=Tanh)

    # --- stage inputs; kn/qn gate the setup. Chunked DMAs as separate
    # tiles across both HWDGE rings so each transpose starts as soon as its
    # own chunk lands ---
    kr = k_d.rearrange("(c j) k -> j c k", c=4)
    kn = []
    for cj in range(4):
        t = setup.tile([P, QK], FP32, name=f"kn{cj}", tag=f"kn{cj}")
        (nc.sync if cj % 2 == 0 else nc.scalar).dma_start(out=t, in_=kr[:, cj])
        kn.append(t)
    qr = q_d.rearrange("(c i) k -> i c k", c=2)
    qn = []
    for ci in range(2):
        t = setup.tile([P, QK], FP32, name=f"qn{ci}", tag=f"qn{ci}")
        (nc.sync if ci % 2 == 0 else nc.scalar).dma_start(out=t, in_=qr[:, ci])
        qn.append(t)

    # --- transpose key to keyT [k, cj*128+j]; copies alternate DVE/ACT ---
    keyT = setup.tile([P, 2, M], FP16, name="keyT")  # [k, ck, j]
    for n, (cj, kc) in enumerate([(c, k) for c in range(4) for k in range(2)]):
        tp = ps_tp.tile([P, P], FP32, name="tp", tag="tp")
        nc.tensor.transpose(tp, kn[cj][:, ts(kc, P)], ident)
        if n % 2 == 0:
            nc.vector.tensor_copy(out=keyT[:, kc, ts(cj, P)], in_=tp)
        else:
            nc.scalar.copy(out=keyT[:, kc, ts(cj, P)], in_=tp)

    # --- weights (gpsimd SWDGE cast-DMAs; emitted late so gpsimd builds
    # ident first) ---
    wk_sb = persist.tile([P, 2, H], FP16, name="wk_sb")
    nc.gpsimd.dma_start(out=wk_sb, in_=wk_d.rearrange("(c k) h -> k c h", c=2))
    wq_sb = persist.tile([P, 2, H], FP16, name="wq_sb")
    nc.gpsimd.dma_start(out=wq_sb, in_=wq_d.rearrange("(c k) h -> k c h", c=2))
    wv_sb = persist.tile([P, 1], FP16, name="wv_sb")
    nc.gpsimd.dma_start(out=wv_sb, in_=wv_d)  # casts fp32 -> fp16

    # --- project: kT = W_k^T @ key^T  [h, j] ---
    kt_ps = ps_one.tile([P, M], FP32, name="kt_ps", tag="proj")
    for kc in range(2):
        nc.tensor.matmul(kt_ps, lhsT=wk_sb[:, kc, :], rhs=keyT[:, kc, :],
                         start=(kc == 0), stop=(kc == 1))
    sdt = FP16 if S_FP16 else FP32
    kT_sb = persist.tile([P, M], sdt, name="kT_sb")
    nc.vector.tensor_copy(out=kT_sb, in_=kt_ps)

    # --- transpose query to queryT [k, ci*128+i] ---
    queryT = setup.tile([P, 2, N_LOC], FP16, name="queryT")  # [k, ck, i]
    for n, (ci, kc) in enumerate([(c, k) for c in range(2) for k in range(2)]):
        tp = ps_tp.tile([P, P], FP32, name="tp", tag="tp")
        nc.tensor.transpose(tp, qn[ci][:, ts(kc, P)], ident)
        if n % 2 == 0:
            nc.vector.tensor_copy(out=queryT[:, kc, ts(ci, P)], in_=tp)
        else:
            nc.scalar.copy(out=queryT[:, kc, ts(ci, P)], in_=tp)

    # --- project: qT = W_q^T @ query^T  [h, i] ---
    qt_ps = ps_one.tile([P, N_LOC], FP32, name="qt_ps", tag="proj")
    for kc in range(2):
        nc.tensor.matmul(qt_ps, lhsT=wq_sb[:, kc, :], rhs=queryT[:, kc, :],
                         start=(kc == 0), stop=(kc == 1))
    # qT feeds tensor_scalar's scalar1 operand, which must be fp32
    qT_sb = persist.tile([P, N_LOC], FP32, name="qT_sb")
    nc.scalar.copy(out=qT_sb, in_=qt_ps)

    # value: cast-DMA (SWDGE) straight into fp16; ones column = softmax denom
    v_hf = persist.tile([P, 4, DV + 1], FP16, name="v_hf")
    nc.gpsimd.dma_start(out=v_hf[:, :, 0:DV],
                        in_=v_d.rearrange("(c j) d -> j c d", c=4))
    nc.vector.memset(v_hf[:, :, DV:DV + 1], 1.0)

    # --- scoresT PSUM tile: [j % 128, cj, i] (4KB/partition = 2 banks) ---
    scT = ps_sc.tile([P, 4, N_LOC], FP32, name="scT", tag="scT")

    e_T = persist.tile([P, 4, N_LOC], FP16, name="e_T")

    def finish_half(ic):
        # exp of scoresT columns [ic*128, (ic+1)*128) (no max subtraction:
        # |scores| <= sum|W_v| < 10), then attn @ value (+ones col =
        # denominators), normalize, store. For ic=0 this is emitted
        # mid-loop so it hides under the remaining tanh blocks.
        nc.scalar.activation(out=e_T[:, :, ts(ic, P)], in_=scT[:, :, ts(ic, P)],
                             func=Exp)
        o_ps = ps_one.tile([P, DV + 1], FP32, name="o_ps", tag="o_ps")
        for cj in range(4):
            nc.tensor.matmul(o_ps, lhsT=e_T[:, cj, ts(ic, P)],
                             rhs=v_hf[:, cj, :],
                             start=(cj == 0), stop=(cj == 3))
        rec = outp.tile([P, 1], FP32, name="rec", tag="rec")
        nc.vector.reciprocal(rec, o_ps[:, DV:DV + 1])
        o_sb = outp.tile([P, DV], FP32, name="o_sb", tag="o_sb")
        nc.vector.tensor_scalar_mul(out=o_sb, in0=o_ps[:, 0:DV], scalar1=rec)
        nc.sync.dma_start(out=out_d[ts(ic, P), :], in_=o_sb)

    # --- main loop: tanh features + W_v reduction ---
    i0 = 0
    for blk, nb in enumerate(BLOCKS):
        s = s_pool.tile([P, nb * M], sdt, name="s", tag="s")
        for ii in range(nb):
            i = i0 + ii
            nc.vector.tensor_scalar_add(
                out=s[:, ts(ii, M)], in0=kT_sb, scalar1=qT_sb[:, i:i + 1])
        f = f_pool.tile([P, nb * M], FP16, name="f", tag="f")
        nc.scalar.activation(out=f, in_=s, func=Tanh)
        for ii in range(nb):
            i = i0 + ii
            for cj in range(4):
                nc.tensor.matmul(
                    scT[:, cj, i:i + 1],
                    lhsT=f[:, ii * M + cj * P: ii * M + (cj + 1) * P],
                    rhs=wv_sb, start=True, stop=True)
        i0 += nb
        if blk == HALF_BLK + 1:
            finish_half(0)

    finish_half(1)


def _build_nc():
    nc = bacc.Bacc("TRN2", target_bir_lowering=False, debug=False, num_devices=8)
    q_d = nc.dram_tensor("query", [N_LOC, QK], FP32, kind="ExternalInput").ap()
    k_d = nc.dram_tensor("key", [M, QK], FP32, kind="ExternalInput").ap()
    v_d = nc.dram_tensor("value", [M, DV], FP32, kind="ExternalInput").ap()
    wq_d = nc.dram_tensor("W_q", [QK, H], FP32, kind="ExternalInput").ap()
    wk_d = nc.dram_tensor("W_k", [QK, H], FP32, kind="ExternalInput").ap()
    wv_d = nc.dram_tensor("W_v", [H, 1], FP32, kind="ExternalInput").ap()
    out_d = nc.dram_tensor("out", [N_LOC, DV], FP32, kind="ExternalOutput").ap()
    with tile.TileContext(nc) as tc:
        with ExitStack() as ctx:
            _body(tc, q_d, k_d, v_d, wq_d, wk_d, wv_d, out_d, ctx)
    nc.compile()
    return nc


def get_nc():
    global _NC
    if _NC is None:
        _NC = _build_nc()
    return _NC


def make_in_maps(query, key, value, W_q, W_k, W_v):
    query = np.ascontiguousarray(query, dtype=np.float32)
    key = np.ascontiguousarray(key, dtype=np.float32)
    value = np.ascontiguousarray(value, dtype=np.float32)
    W_q = np.ascontiguousarray(W_q, dtype=np.float32)
    W_k = np.ascontiguousarray(W_k, dtype=np.float32)
    W_v = np.ascontiguousarray(W_v, dtype=np.float32).reshape(H, 1)
    in_maps = []
    for core in range(8):
        b, half = divmod(core, 2)
        in_maps.append({
            "query": query[b, half * N_LOC:(half + 1) * N_LOC, :],
            "key": key[b],
            "value": value[b],
            "W_q": W_q,
            "W_k": W_k,
            "W_v": W_v,
        })
    return in_maps


def kernel(query, key, value, W_q, W_k, W_v):
    global LAST_RESULT
    nc = get_nc()
    in_maps = make_in_maps(query, key, value, W_q, W_k, W_v)
    trace = os.environ.get("BASS_TRACE", "") == "1"
    res = run_bass_kernel_spmd(nc, in_maps, core_ids=list(range(8)), trace=trace)
    LAST_RESULT = res
    out = np.empty((4, 512, DV), dtype=np.float32)
    for core in range(8):
        b, half = divmod(core, 2)
        out[b, half * N_LOC:(half + 1) * N_LOC, :] = res.results[core]["out"]
    return out
